# revision 15
# baseline (speedup 1.0000x reference)
"""Trainium2 Bass kernel for a custom LSTM cell step.

Reference computation (per full problem, B=8192, D=U=512):
    z = inputs @ kernel + h_tm1 @ recurrent_kernel + bias        # [B, 4U]
    i, f, g, o = split(z, 4, axis=1)
    i, f, o = sigmoid(...)  ;  g = tanh(g)
    c = f * c_tm1 + i * g
    h = o * tanh(c)
    return (h, h, c)

Sharding: data-parallel over the batch dim across 8 NeuronCores
(1024 rows per core); kernel/recurrent_kernel/bias replicated.

Design (bf16, PE-bound at ~55us/core):
  - All matmul operands are cast to bf16 on the host (measured end-to-end
    rel err ~3e-3 vs the 2e-2 gate; fp8 measured 2.7e-2+ and is ruled out).
  - x and h are transposed and stacked on the host into xh_t [D+U, MB]
    per core, so the PE does zero transposes: the stationary operand
    tiles [128k, 128m] come straight from DRAM.
  - One stacked [1024, 2048] contraction (k-tiles 0-3 = W fed by x rows,
    4-7 = R fed by h rows); 8 m-tiles x 4 gates x 8 k = 256 matmuls of
    N=512, each 213ns warm -> ~54.6us dense PE.
  - The PE clock is HAM-throttled to 1.2 GHz until ~3.4us of sustained
    busy; a burst of warmup matmuls on a zeroed tile at t=0 rides out
    the throttle window while the first DMAs land, so all real matmuls
    run at 2.4 GHz.
  - Inputs stream on both HWDGE rings (weights on sync, activations on
    scalar) in arrival-paced chunks; outputs (bf16) leave via gpsimd.
  - c_tm1 / c_out / h_out are bf16 on the wire; elementwise math and
    PSUM accumulation stay fp32.
"""

from contextlib import ExitStack

import numpy as np
import ml_dtypes

import concourse.bass as bass
import concourse.mybir as mybir
import concourse.tile as tile
from concourse import bacc
from concourse.bass_utils import run_bass_kernel_spmd

# Problem sizes (hardcoded per spec).
B, D, U = 8192, 512, 512
N_CORES = 8
MB = B // N_CORES  # 1024 batch rows per core
P = 128
MT = MB // P  # 8 m-tiles per core
KO = (D + U) // P  # 8 stacked contraction tiles (4 from W/x, 4 from R/h)
NG = 4 * U  # 2048 gate columns

F32 = mybir.dt.float32
BF16 = mybir.dt.bfloat16
NPBF16 = ml_dtypes.bfloat16

SIG = mybir.ActivationFunctionType.Sigmoid
TANH = mybir.ActivationFunctionType.Tanh

N_WARMUP = 28  # small warmup matmuls bridging engine-start to first data

_NC_CACHE: dict = {}


def _build_lstm_nc(with_bias: bool):
    """Build and compile the per-core Bass program."""
    nc = bacc.Bacc("TRN2", target_bir_lowering=False, debug=False)

    xhT_d = nc.dram_tensor("xh_t", [D + U, MB], BF16, kind="ExternalInput")
    c_d = nc.dram_tensor("c_tm1", [MB, U], BF16, kind="ExternalInput")
    w_d = nc.dram_tensor("kernel", [D, NG], BF16, kind="ExternalInput")
    r_d = nc.dram_tensor("recurrent_kernel", [U, NG], BF16, kind="ExternalInput")
    b_d = None
    if with_bias:
        b_d = nc.dram_tensor("bias", [NG], F32, kind="ExternalInput")
    ho_d = nc.dram_tensor("h_out", [MB, U], BF16, kind="ExternalOutput")
    co_d = nc.dram_tensor("c_out", [MB, U], BF16, kind="ExternalOutput")

    # DRAM views tiled to [partition, tile, free]
    xhT_v = xhT_d.ap().rearrange("(ko p) m -> p ko m", p=P)  # [128, 8, 1024]
    c_v = c_d.ap().rearrange("(mt p) u -> p mt u", p=P)
    ho_v = ho_d.ap().rearrange("(mt p) u -> p mt u", p=P)
    co_v = co_d.ap().rearrange("(mt p) u -> p mt u", p=P)
    w_v = w_d.ap().rearrange("(ko p) n -> p ko n", p=P)  # [128, 4, 2048]
    r_v = r_d.ap().rearrange("(ko p) n -> p ko n", p=P)

    # Gate -> z column slice (reference split order: i, f, g, o).
    GATE_COLS = {"i": slice(0, U), "f": slice(U, 2 * U),
                 "g": slice(2 * U, 3 * U), "o": slice(3 * U, 4 * U)}

    with tile.TileContext(nc) as tc, ExitStack() as ctx:
        consts = ctx.enter_context(tc.tile_pool(name="consts", bufs=1))
        gi = ctx.enter_context(tc.tile_pool(name="gi", bufs=MT))
        gig = ctx.enter_context(tc.tile_pool(name="gig", bufs=MT))
        gth = ctx.enter_context(tc.tile_pool(name="gth", bufs=MT))
        scratch = ctx.enter_context(tc.tile_pool(name="scratch", bufs=3))
        outp = ctx.enter_context(tc.tile_pool(name="outp", bufs=4))
        zpsum = ctx.enter_context(tc.tile_pool(name="zpsum", bufs=7, space="PSUM"))
        warm = ctx.enter_context(tc.tile_pool(name="warm", bufs=1, space="PSUM"))

        xhT = consts.tile([P, KO, MB], BF16)      # 16KB/partition
        wr = consts.tile([P, KO, NG], BF16)       # 32KB/partition
        c_sb = consts.tile([P, MT, U], BF16)      # 8KB/partition
        wu = consts.tile([P, P], BF16)            # warmup operand

        # --- PE warmup: keep the PE busy from engine-start until the first
        # data lands, so the HAM clock-gate is (nearly) warm for real work.
        nc.vector.memset(wu[:], 0.0)
        wt = warm.tile([P, P], F32)
        for _ in range(N_WARMUP):
            nc.tensor.matmul(wt[:], wu[:], wu[:], start=True, stop=True)

        # --- input DMA, all on the sync HWDGE ring, arrival-paced ---
        def load_xhT(ks, ms):
            nc.sync.dma_start(xhT[:, ks, ms], xhT_v[:, ks, ms])

        def load_w(gate):
            sl = GATE_COLS[gate]
            nc.sync.dma_start(wr[:, 0:4, sl], w_v[:, :, sl])
            nc.sync.dma_start(wr[:, 4:8, sl], r_v[:, :, sl])

        Q = MB // 4
        ICOLS = GATE_COLS["i"]
        # The first accumulation group needs Wi + Ri + xhT[mt0-1]. Each DMA
        # ring pays a ~2.5us head latency before its first bytes land, so
        # the three pieces go on three different rings and arrive in
        # parallel at ~10.5us:
        #   sync (HWDGE):   Wi, then the rest of xhT, W/R, outputs
        #   scalar (HWDGE): xhT[mt0-1] (its only DMA; done before ACTs start)
        #   gpsimd (SWDGE): Ri, then c
        nc.sync.dma_start(wr[:, 0:4, ICOLS], w_v[:, :, ICOLS])       # Wi
        nc.scalar.dma_start(xhT[:, :, 0:Q], xhT_v[:, :, 0:Q])    # mt0-1, all k
        nc.gpsimd.dma_start(wr[:, 4:8, ICOLS], r_v[:, :, ICOLS])     # Ri
        load_xhT(slice(0, 8), slice(Q, 2 * Q))        # mt2-3 (0.5MB, sync)
        load_xhT(slice(0, 8), slice(2 * Q, 3 * Q))
        load_xhT(slice(0, 8), slice(3 * Q, 4 * Q))
        nc.gpsimd.dma_start(c_sb[:], c_v)
        load_w("g")
        load_w("f")
        load_w("o")

        bias_bc = None
        if with_bias:
            assert b_d is not None
            bias_bc = consts.tile([P, NG], F32)
            b_ap = b_d.ap()
            # DMA-replicate bias across all 128 partitions (partition step 0).
            nc.gpsimd.dma_start(
                out=bias_bc,
                in_=bass.AP(tensor=b_ap.tensor, offset=b_ap.offset, ap=[[0, P], [1, NG]]),
            )

        def z_chunk(gate, mt):
            """Accumulate z[:, gate cols] for m-tile mt into a PSUM bank."""
            sl = GATE_COLS[gate]
            zp = zpsum.tile([P, U], F32, tag="z")
            msl = slice(mt * P, (mt + 1) * P)
            for ko in range(KO):
                nc.tensor.matmul(
                    zp[:],
                    xhT[:, ko, msl],
                    wr[:, ko, sl],
                    start=(ko == 0),
                    stop=(ko == KO - 1),
                )
            if bias_bc is not None:
                nc.vector.tensor_add(zp[:], zp[:], bias_bc[:, sl])
            return zp

        i_t, ig_t, th_t = {}, {}, {}

        def phase_i_pair01():
            """First two i-groups, k-interleaved across two PSUM banks so
            the supply-paced arrival of each k chunk unlocks work for both
            m-tiles (the PE fills the HBM-bound startup with real MMs)."""
            zps = [zpsum.tile([P, U], F32, tag="z", name=f"zp0{m}") for m in (0, 1)]
            for kk in (0, 2, 4, 6):
                for m in (0, 1):
                    for ko in (kk, kk + 1):
                        nc.tensor.matmul(
                            zps[m][:],
                            xhT[:, ko, m * P : (m + 1) * P],
                            wr[:, ko, ICOLS],
                            start=(ko == 0),
                            stop=(ko == KO - 1),
                        )
            for m in (0, 1):
                if bias_bc is not None:
                    nc.vector.tensor_add(zps[m][:], zps[m][:], bias_bc[:, ICOLS])
                it = gi.tile([P, U], F32, tag="i", name=f"it0{m}")
                nc.scalar.activation(it[:], zps[m][:], SIG)
                i_t[m] = it

        def phase_i(mt):  # i = sigmoid(z0)
            it = gi.tile([P, U], F32, tag="i")
            nc.scalar.activation(it[:], z_chunk("i", mt)[:], SIG)
            i_t[mt] = it

        def phase_g(mt):  # g = tanh(z2); ig = i*g
            gt = scratch.tile([P, U], F32, tag="gact")
            nc.scalar.activation(gt[:], z_chunk("g", mt)[:], TANH)
            ig = gig.tile([P, U], F32, tag="ig")
            nc.vector.tensor_mul(ig[:], i_t.pop(mt)[:], gt[:])
            ig_t[mt] = ig

        c_pair, h_pair = {}, {}

        def phase_f(mt):  # f = sigmoid(z1); c = f*c_old + ig; tanh(c)
            ft = scratch.tile([P, U], F32, tag="gact")
            nc.scalar.activation(ft[:], z_chunk("f", mt)[:], SIG)
            fc = scratch.tile([P, U], F32, tag="fc")
            nc.vector.tensor_mul(fc[:], ft[:], c_sb[:, mt, :])
            if mt % 2 == 0:
                c_pair[mt // 2] = outp.tile([P, 2, U], BF16, tag="cnew", name=f"c_pair{mt // 2}")
            c_new = c_pair[mt // 2]
            nc.vector.tensor_add(c_new[:, mt % 2, :], fc[:], ig_t.pop(mt)[:])
            if mt % 2 == 1:  # pair complete -> one 0.25MB store on the fast ring
                nc.sync.dma_start(co_v[:, mt - 1 : mt + 1, :], c_pair[mt // 2][:])
            th = gth.tile([P, U], F32, tag="th")
            nc.scalar.activation(th[:], c_new[:, mt % 2, :], TANH)
            th_t[mt] = th

        def phase_o(mt):  # o = sigmoid(z3); h = o*tanh(c)
            if mt == MT - 1:
                return phase_o_last(mt)
            ot = scratch.tile([P, U], F32, tag="gact")
            nc.scalar.activation(ot[:], z_chunk("o", mt)[:], SIG)
            if mt == MT - 2:  # penultimate: own store so the tail stays short
                h_new = outp.tile([P, U], BF16, tag="hnew", name="h_penult")
                nc.vector.tensor_mul(h_new[:], ot[:], th_t.pop(mt)[:])
                nc.sync.dma_start(ho_v[:, mt, :], h_new[:])
                return
            if mt % 2 == 0:
                h_pair[mt // 2] = outp.tile([P, 2, U], BF16, tag="hnew", name=f"h_pair{mt // 2}")
            nc.vector.tensor_mul(h_pair[mt // 2][:, mt % 2, :], ot[:], th_t.pop(mt)[:])
            if mt % 2 == 1:
                nc.sync.dma_start(ho_v[:, mt - 1 : mt + 1, :], h_pair[mt // 2][:])

        def phase_o_last(mt):
            """Last group: half-column granularity so the ACT/DVE/DMA tail
            chain after the final matmul is halved, and the last store is
            a small single-tile DMA."""
            sl = GATE_COLS["o"]
            NQ = U // 4
            h_new = outp.tile([P, U], BF16, tag="hnew", name="h_last")
            th = th_t.pop(mt)
            for q in range(4):
                hsl = slice(sl.start + q * NQ, sl.start + (q + 1) * NQ)
                usl = slice(q * NQ, (q + 1) * NQ)
                zp = zpsum.tile([P, NQ], F32, tag="z", name=f"zp_last{q}")
                for ko in range(KO):
                    nc.tensor.matmul(
                        zp[:],
                        xhT[:, ko, mt * P : (mt + 1) * P],
                        wr[:, ko, hsl],
                        start=(ko == 0),
                        stop=(ko == KO - 1),
                    )
                if bias_bc is not None:
                    nc.vector.tensor_add(zp[:], zp[:], bias_bc[:, hsl])
                ot = scratch.tile([P, NQ], F32, tag="gact", name=f"ot_last{q}")
                nc.scalar.activation(ot[:], zp[:], SIG)
                nc.vector.tensor_mul(h_new[:, usl], ot[:], th[:, usl])
            nc.sync.dma_start(ho_v[:, mt, :], h_new[:])

        # Emission order matched to DMA arrivals; PE dense end to end.
        phase_i_pair01()
        for mt in range(2, MT):
            phase_i(mt)
        for mt in range(MT):
            phase_g(mt)
        for mt in range(MT):
            phase_f(mt)
        for mt in range(MT):
            phase_o(mt)

    nc.compile()
    return nc


def _get_nc(with_bias: bool):
    if with_bias not in _NC_CACHE:
        _NC_CACHE[with_bias] = _build_lstm_nc(with_bias)
    return _NC_CACHE[with_bias]


def _prep_in_maps(inputs, h_tm1, c_tm1, kernel, recurrent_kernel, bias):
    """Host-side prep: bf16 casts, stacked transpose, per-core shards."""
    x = np.asarray(inputs, dtype=np.float32)
    h = np.asarray(h_tm1, dtype=np.float32)
    c = np.asarray(c_tm1, dtype=np.float32)
    w16 = np.ascontiguousarray(np.asarray(kernel, dtype=np.float32).astype(NPBF16))
    r16 = np.ascontiguousarray(
        np.asarray(recurrent_kernel, dtype=np.float32).astype(NPBF16)
    )
    b = np.ascontiguousarray(np.asarray(bias, dtype=np.float32))

    xhT = np.empty((D + U, B), dtype=NPBF16)
    xhT[:D] = x.T
    xhT[D:] = h.T
    c16 = c.astype(NPBF16)

    with_bias = bool(np.any(b))
    in_maps = []
    for core in range(N_CORES):
        sl = slice(core * MB, (core + 1) * MB)
        m = {
            "xh_t": np.ascontiguousarray(xhT[:, sl]),
            "c_tm1": np.ascontiguousarray(c16[sl]),
            "kernel": w16,
            "recurrent_kernel": r16,
        }
        if with_bias:
            m["bias"] = b
        in_maps.append(m)
    return in_maps, with_bias


def kernel(inputs, h_tm1, c_tm1, kernel, recurrent_kernel, bias):
    in_maps, with_bias = _prep_in_maps(
        inputs, h_tm1, c_tm1, kernel, recurrent_kernel, bias
    )
    nc = _get_nc(with_bias)
    res = run_bass_kernel_spmd(nc, in_maps, core_ids=list(range(N_CORES)))
    h_out = np.concatenate(
        [np.asarray(r_["h_out"], dtype=np.float32) for r_ in res.results], axis=0
    )
    c_out = np.concatenate(
        [np.asarray(r_["c_out"], dtype=np.float32) for r_ in res.results], axis=0
    )
    return (h_out, h_out, c_out)
